# revision 8
# baseline (speedup 1.0000x reference)
"""Trainium2 Bass kernel for GazeKLDUnit loss.

reference:
    pred_means = pred[:, :2]              # [B, 2]
    true_means = true.mean(axis=1)        # [B, 2]  (mean over T=50)
    kld = 0.5 * sum((true_means - pred_means)**2, -1)   # [B]
    out = mean(kld)                       # scalar

Strategy: data-parallel over 8 cores (B/8 rows each). The problem is pure
HBM bandwidth, so bytes are the lever: host casts to bf16 (final-scalar
quantization error ~4e-7, gate 2e-2), halving DMA. Measured on HW, the DVE
tensor-reduce runs at 1 elem/cycle/lane regardless of dtype/layout, but
tensor_tensor runs at 2 elem/cycle on packed bf16 — so each row's T=50
samples are packed host-side as [t2=2, k, c, t0=25] and the kernel first
folds the two t2 halves with one contiguous tensor_add (2x mode), then
reduces the remaining 25-element segments (1x). That cuts vector time per
tile from 6.7us to ~5us, just under the 4.8us DMA stream time per tile.
Each core streams 16 [128, 6400] bf16 tiles (1.64MB per DMA), subtracts
host-prescaled pred (T*pred), squares and row-reduces to a [128, 1] f32
partial. Host sums partials in float64 and applies 0.5 / (T^2 * B).
"""

import ml_dtypes
import numpy as np

import concourse.bass as bass
import concourse.mybir as mybir
from concourse.bass_utils import run_bass_kernel_spmd

BF16 = ml_dtypes.bfloat16

N_CORES = 8
B = 1048576
T = 50
TH = T // 2                # 25: reduced segment length after the fold
BS = B // N_CORES          # 131072 rows per core
K = 64                     # rows of `true` per partition per tile
TILE_ROWS = 128 * K        # 8192 rows per tile
N_TILES = BS // TILE_ROWS  # 16 tiles per core
F = K * 2 * T              # 6400 bf16 per partition per tile (12.8KB)
FH = F // 2                # 3200: folded tile size
MW = K * 2                 # 128 sums per partition per tile
W = N_TILES * MW           # 2048 sums per partition per core

NBUF = 6

# tile -> folding engine: 1 = Pool (GpSimd), 0 = DVE.  Pool folds are ~3.6x
# slower than DVE folds but run in parallel; DVE keeps all the reduces.
# 10 Pool / 6 DVE balances DVE(6 folds + 16 reduces) against Pool(10 folds).
TILE_ON_POOL = [1, 0, 1, 1, 0, 1, 1, 0, 1, 0, 1, 1, 0, 1, 1, 0]
_ORD = []
_c = {0: 0, 1: 0}
for _e in TILE_ON_POOL:
    _c[_e] += 1
    _ORD.append(_c[_e])
N_POOL = _c[1]
N_DVE = _c[0]

_nc_cache = {}


def _build():
    bf = mybir.dt.bfloat16
    f32 = mybir.dt.float32
    nc = bass.Bass()
    t_in = nc.dram_tensor("t", [N_TILES, 128, F], bf, kind="ExternalInput")
    p_in = nc.dram_tensor("p", [128, W], bf, kind="ExternalInput")
    o_out = nc.dram_tensor("o", [128, 1], f32, kind="ExternalOutput")

    n_dma = N_TILES + 2  # stream tiles + pred + output store

    with (
        nc.allow_low_precision(
            reason="bf16 partials; final scalar mean tolerates ~1e-4"
        ),
        nc.Block() as block,
        nc.semaphore("dma_sem") as dma_sem,
        nc.semaphore("vec_sem") as vec_sem,
        nc.semaphore("vf_sem") as vf_sem,
        nc.semaphore("pf_sem") as pf_sem,
        nc.semaphore("rp_sem") as rp_sem,
        nc.sbuf_tensor("tt", [128, NBUF * F], bf) as tt,
        nc.sbuf_tensor("fold", [128, FH], bf) as fold,
        nc.sbuf_tensor("fold_p", [128, 2 * FH], bf) as fold_p,
        nc.sbuf_tensor("pred_t", [128, W], bf) as pred_t,
        nc.sbuf_tensor("msum", [128, W], bf) as msum,
        nc.sbuf_tensor("dbuf", [128, W], bf) as dbuf,
        nc.sbuf_tensor("d2buf", [128, W], bf) as d2buf,
        nc.sbuf_tensor("accb", [128, 1], f32) as accb,
    ):

        @block.sync
        def _(sync):
            for i in range(N_TILES):
                slot = i % NBUF
                if i >= NBUF:
                    # slot reuse: previous occupant's fold must have consumed it
                    prev = i - NBUF
                    sem = pf_sem if TILE_ON_POOL[prev] else vf_sem
                    sync.wait_ge(sem, _ORD[prev])
                sync.dma_start(
                    tt[:, slot * F : (slot + 1) * F], t_in[i]
                ).then_inc(dma_sem, 16)
            sync.dma_start(pred_t[:, :], p_in[:, :]).then_inc(dma_sem, 16)
            sync.wait_ge(vec_sem, 2)
            sync.dma_start(o_out[:, :], accb[:, :]).then_inc(dma_sem, 16)
            sync.wait_ge(dma_sem, 16 * n_dma)

        @block.vector
        def _(vector):
            for i in range(N_TILES):
                slot = i % NBUF
                if TILE_ON_POOL[i]:
                    j = _ORD[i]  # 1-based pool-fold ordinal
                    vector.wait_ge(pf_sem, j)
                    ps = (j - 1) % 2
                    src = fold_p[:, ps * FH : (ps + 1) * FH]
                else:
                    vector.wait_ge(dma_sem, 16 * (i + 1))
                    a = tt[:, slot * F : slot * F + FH]
                    b = tt[:, slot * F + FH : (slot + 1) * F]
                    # 2x-mode contiguous fold of the two t2 halves
                    vector.tensor_add(fold[:, :], a, b).then_inc(vf_sem, 1)
                    src = fold[:, :]
                v = src.rearrange("p (s t) -> p s t", s=MW, t=TH)
                red = vector.reduce_sum(
                    msum[:, i * MW : (i + 1) * MW], v, axis=mybir.AxisListType.X
                )
                if TILE_ON_POOL[i]:
                    red.then_inc(rp_sem, 1)
            vector.wait_ge(dma_sem, 16 * (N_TILES + 1))  # pred loaded
            vector.tensor_sub(dbuf[:, :], msum[:, :], pred_t[:, :]).then_inc(
                vec_sem, 1
            )
            vector.tensor_mul(d2buf[:, :], dbuf[:, :], dbuf[:, :])
            vector.reduce_sum(
                accb[:, :], d2buf[:, :], axis=mybir.AxisListType.X
            ).then_inc(vec_sem, 1)

        @block.gpsimd
        def _(gpsimd):
            jj = 0  # 0-based pool-fold ordinal
            for i in range(N_TILES):
                if not TILE_ON_POOL[i]:
                    continue
                gpsimd.wait_ge(dma_sem, 16 * (i + 1))
                if jj >= 2:
                    # fold slot reuse: DVE must have reduced fold jj-2
                    gpsimd.wait_ge(rp_sem, jj - 1)
                slot = i % NBUF
                ps = jj % 2
                a = tt[:, slot * F : slot * F + FH]
                b = tt[:, slot * F + FH : (slot + 1) * F]
                gpsimd.tensor_add(
                    fold_p[:, ps * FH : (ps + 1) * FH], a, b
                ).then_inc(pf_sem, 1)
                jj += 1

    return nc


def _prep_inputs(pred, true):
    """Per-core input maps: shard + cast bf16 + [t2, k, c, t0] pack."""
    true_bf = np.asarray(true).astype(BF16)                      # [B, T, 2]
    pred50 = (np.asarray(pred[:, :2]) * np.float32(T)).astype(BF16)  # [B, 2]
    in_maps = []
    for c in range(N_CORES):
        sh = true_bf[c * BS : (c + 1) * BS]                      # [BS, T, 2]
        # [i, p, k, t2, t0, c] -> [i, p, t2, k, c, t0]
        t_shard = np.ascontiguousarray(
            sh.reshape(N_TILES, 128, K, 2, TH, 2).transpose(0, 1, 3, 2, 5, 4)
        ).reshape(N_TILES, 128, F)
        p_shard = (
            pred50[c * BS : (c + 1) * BS]
            .reshape(N_TILES, 128, K, 2)
            .transpose(1, 0, 2, 3)
            .reshape(128, W)
        )
        in_maps.append({"t": t_shard, "p": np.ascontiguousarray(p_shard)})
    return in_maps


def _finish(results):
    total = np.float64(0.0)
    for r in results:
        total += r["o"].astype(np.float64).sum()
    val = total * 0.5 / (T * T) / B
    return np.array(val, dtype=np.float32)


def _get_nc():
    if "nc" not in _nc_cache:
        _nc_cache["nc"] = _build()
    return _nc_cache["nc"]


def kernel(pred, true):
    nc = _get_nc()
    in_maps = _prep_inputs(pred, true)
    res = run_bass_kernel_spmd(nc, in_maps, list(range(N_CORES)))
    return _finish(res.results)


def kernel_traced(pred, true, **trace_kwargs):
    nc = _get_nc()
    in_maps = _prep_inputs(pred, true)
    res = run_bass_kernel_spmd(
        nc, in_maps, list(range(N_CORES)), trace=True, **trace_kwargs
    )
    return _finish(res.results), res


# revision 9
# speedup vs baseline: 1.0340x; 1.0340x over previous
"""Trainium2 Bass kernel for GazeKLDUnit loss.

reference:
    pred_means = pred[:, :2]              # [B, 2]
    true_means = true.mean(axis=1)        # [B, 2]  (mean over T=50)
    kld = 0.5 * sum((true_means - pred_means)**2, -1)   # [B]
    out = mean(kld)                       # scalar

Strategy: data-parallel over 8 cores (B/8 rows each). The problem is pure
HBM bandwidth, so bytes are the lever: host casts to bf16 (final-scalar
quantization error ~4e-7, gate 2e-2), halving DMA. Measured on HW, the DVE
tensor-reduce runs at 1 elem/cycle/lane regardless of dtype/layout, but
tensor_tensor runs at 2 elem/cycle on packed bf16 — so each row's T=50
samples are packed host-side as [t2=2, k, c, t0=25] and the kernel first
folds the two t2 halves with one contiguous tensor_add (2x mode), then
reduces the remaining 25-element segments (1x). That cuts vector time per
tile from 6.7us to ~5us, just under the 4.8us DMA stream time per tile.
Each core streams 16 [128, 6400] bf16 tiles (1.64MB per DMA), subtracts
host-prescaled pred (T*pred), squares and row-reduces to a [128, 1] f32
partial. Host sums partials in float64 and applies 0.5 / (T^2 * B).
"""

import ml_dtypes
import numpy as np

import concourse.bass as bass
import concourse.mybir as mybir
from concourse.bass_utils import run_bass_kernel_spmd

BF16 = ml_dtypes.bfloat16

N_CORES = 8
B = 1048576
T = 50
TH = T // 2                # 25: reduced segment length after the fold
BS = B // N_CORES          # 131072 rows per core
K = 64                     # rows of `true` per partition per tile
TILE_ROWS = 128 * K        # 8192 rows per tile
N_TILES = BS // TILE_ROWS  # 16 tiles per core
F = K * 2 * T              # 6400 bf16 per partition per tile (12.8KB)
FH = F // 2                # 3200: folded tile size
MW = K * 2                 # 128 sums per partition per tile
W = N_TILES * MW           # 2048 sums per partition per core

NBUF = 6

_nc_cache = {}


def _build():
    bf = mybir.dt.bfloat16
    f32 = mybir.dt.float32
    nc = bass.Bass()
    t_in = nc.dram_tensor("t", [N_TILES, 128, F], bf, kind="ExternalInput")
    p_in = nc.dram_tensor("p", [128, W], bf, kind="ExternalInput")
    o_out = nc.dram_tensor("o", [128, 1], f32, kind="ExternalOutput")

    n_dma = N_TILES + 2  # stream tiles + pred + output store

    with (
        nc.allow_low_precision(
            reason="bf16 partials; final scalar mean tolerates ~1e-4"
        ),
        nc.Block() as block,
        nc.semaphore("dma_sem") as dma_sem,
        nc.semaphore("vec_sem") as vec_sem,
        nc.sbuf_tensor("tt", [128, NBUF * F], bf) as tt,
        nc.sbuf_tensor("fold", [128, FH], bf) as fold,
        nc.sbuf_tensor("pred_t", [128, W], bf) as pred_t,
        nc.sbuf_tensor("msum", [128, W], bf) as msum,
        nc.sbuf_tensor("dbuf", [128, W], bf) as dbuf,
        nc.sbuf_tensor("d2buf", [128, W], bf) as d2buf,
        nc.sbuf_tensor("accb", [128, 1], f32) as accb,
    ):

        @block.sync
        def _(sync):
            for i in range(N_TILES):
                slot = i % NBUF
                if i >= NBUF:
                    # slot reuse: previous occupant's fold must have consumed it
                    sync.wait_ge(vec_sem, i - NBUF + 1)
                sync.dma_start(
                    tt[:, slot * F : (slot + 1) * F], t_in[i]
                ).then_inc(dma_sem, 16)
            sync.dma_start(pred_t[:, :], p_in[:, :]).then_inc(dma_sem, 16)
            sync.wait_ge(vec_sem, N_TILES + 2)
            sync.dma_start(o_out[:, :], accb[:, :]).then_inc(dma_sem, 16)
            sync.wait_ge(dma_sem, 16 * n_dma)

        @block.vector
        def _(vector):
            for i in range(N_TILES):
                vector.wait_ge(dma_sem, 16 * (i + 1))
                slot = i % NBUF
                a = tt[:, slot * F : slot * F + FH]
                b = tt[:, slot * F + FH : (slot + 1) * F]
                # 2x-mode contiguous fold of the two t2 halves
                vector.tensor_add(fold[:, :], a, b).then_inc(vec_sem, 1)
                v = fold[:, :].rearrange("p (s t) -> p s t", s=MW, t=TH)
                vector.reduce_sum(
                    msum[:, i * MW : (i + 1) * MW], v, axis=mybir.AxisListType.X
                )
            vector.wait_ge(dma_sem, 16 * (N_TILES + 1))  # pred loaded
            vector.tensor_sub(dbuf[:, :], msum[:, :], pred_t[:, :]).then_inc(
                vec_sem, 1
            )
            vector.tensor_mul(d2buf[:, :], dbuf[:, :], dbuf[:, :])
            vector.reduce_sum(
                accb[:, :], d2buf[:, :], axis=mybir.AxisListType.X
            ).then_inc(vec_sem, 1)

    return nc


def _prep_inputs(pred, true):
    """Per-core input maps: shard + cast bf16 + [t2, k, c, t0] pack."""
    true_bf = np.asarray(true).astype(BF16)                      # [B, T, 2]
    pred50 = (np.asarray(pred[:, :2]) * np.float32(T)).astype(BF16)  # [B, 2]
    in_maps = []
    for c in range(N_CORES):
        sh = true_bf[c * BS : (c + 1) * BS]                      # [BS, T, 2]
        # [i, p, k, t2, t0, c] -> [i, p, t2, k, c, t0]
        t_shard = np.ascontiguousarray(
            sh.reshape(N_TILES, 128, K, 2, TH, 2).transpose(0, 1, 3, 2, 5, 4)
        ).reshape(N_TILES, 128, F)
        p_shard = (
            pred50[c * BS : (c + 1) * BS]
            .reshape(N_TILES, 128, K, 2)
            .transpose(1, 0, 2, 3)
            .reshape(128, W)
        )
        in_maps.append({"t": t_shard, "p": np.ascontiguousarray(p_shard)})
    return in_maps


def _finish(results):
    total = np.float64(0.0)
    for r in results:
        total += r["o"].astype(np.float64).sum()
    val = total * 0.5 / (T * T) / B
    return np.array(val, dtype=np.float32)


def _get_nc():
    if "nc" not in _nc_cache:
        _nc_cache["nc"] = _build()
    return _nc_cache["nc"]


def kernel(pred, true):
    nc = _get_nc()
    in_maps = _prep_inputs(pred, true)
    res = run_bass_kernel_spmd(nc, in_maps, list(range(N_CORES)))
    return _finish(res.results)


def kernel_traced(pred, true, **trace_kwargs):
    nc = _get_nc()
    in_maps = _prep_inputs(pred, true)
    res = run_bass_kernel_spmd(
        nc, in_maps, list(range(N_CORES)), trace=True, **trace_kwargs
    )
    return _finish(res.results), res
